# revision 21
# baseline (speedup 1.0000x reference)
"""Nonstationary Matern-5/2 kernel matrix on 8 Trainium2 NeuronCores.

Math: out[i,j] = (1 + u + u^2/3) * exp(-u),  u = sqrt5 * r_ij * (s(x_i)+s(y_j))
where r_ij = ||x_i - y_j|| and s() is a tiny MLP (Linear-selu-Linear-softplus).

Key trick: u^2 = 5*r2*S^2 where r2 (rank-5 in outer-product terms) and
S^2 = (sx+sy)^2 (rank-3) multiply elementwise into a rank-15 sum, so ONE
K=16 matmul per output tile produces w = u^2 (+ a constant clamp column).
Then per tile: u = sqrt(w) on ACT, e = exp(-u - ln3) on ACT, and
out = ((u+1.5)^2 + 0.75) * e in a single fused custom DVE op
(since 1+u+u^2/3 = ((u+1.5)^2 + 0.75)/3).

Perf structure (v2):
 - All x/y point handling starts from the packed natural layout
   ([128, 3k] contiguous per partition); coords-on-partitions layouts are
   derived via PE-array transposes, never via 4-byte-granular DMA.
 - Matmuls run as float32r (full PE rate at moving width 512, ~4x fp32).
 - Main loop phases of 4 strips batch all sqrts before all exps so the
   ACT table set switches only twice per phase.
 - exp output is fp16 (halves its SBUF/bandwidth), final out fp32.

Sharding: data-parallel over rows of x; each core computes a [1024, 8192]
block; y and MLP params replicated.
"""

import numpy as np

import concourse.bacc as bacc
import concourse.bass as bass
import concourse.mybir as mybir
from concourse.mybir import AluOpType as alu
from concourse.tile import TileContext
from concourse.bass_utils import run_bass_kernel_spmd

import concourse.dve_ops as dve_ops
from concourse.dve_spec import Spec, Src0, Src1, C0, C1, sq, lower
from concourse.dve_uop import DveOpSpec

N, M, D, L = 8192, 8192, 3, 64
N_CORES = 8
ROWS = N // N_CORES          # 1024 x-rows per core
N_STRIPS = ROWS // 128       # 8 strips of 128 partitions
GRP = 2048                   # supertile free width
N_GRP = M // GRP             # 4 col groups per strip
MMW = 512                    # matmul moving width (>=256 keeps f32r at rate)
PHASE_STRIPS = 3             # max strips per sqrt/exp table phase
PHASE_SIZES = [3, 3, 1, 1]   # shrinking phases so the output DMA tail overlaps
EXP_STRIPWISE = True         # one exp instr per strip (coarse deps beat thrash)

NPTS = ROWS + M              # 9216 points: x-shard + y
HALF = NPTS // 2             # 4608 columns per blockdiag half
YG = M // 128                # 64 y grid k-rows
XG = ROWS // 128             # 8 x grid k-rows

LN3 = float(np.log(3.0))
SELU_L = 1.0507009873554805
SELU_A = 1.6732632423543772
CLAMP_EPS = 5e-3             # w = u^2 clamp floor (vs matmul noise)

F32 = mybir.dt.float32
F16 = mybir.dt.float16
F32R = mybir.dt.float32r
Act = mybir.ActivationFunctionType


def _register_matern_tail():
    """out = ((in0 + s0)^2 + s1) * in1, one fused DVE instruction."""
    name = "MATERN_TAIL_ANT"
    for o in dve_ops.OPS:
        if o.name == name:
            return o
    spec = Spec(
        body=(sq(Src0 + C0) + C1) * Src1,
        reference=lambda in0, in1, s0, s1, imm2: (
            ((in0.astype(np.float32) + s0) ** 2 + s1) * in1.astype(np.float32)
        ).astype(np.float32),
    )
    shas = {}
    for ver in ("v3", "v4"):
        uops = lower(spec, ver=ver)
        shas[ver] = DveOpSpec(name=name, opcode=1, uops=uops, rd1_en=True).sha(ver)
    op = dve_ops.DveOp(name, spec, subdim=False, uops_sha=shas)
    dve_ops.OPS.append(op)
    dve_ops.CUSTOM_DVE_SPECS[name] = spec
    dve_ops._SUB_OPCODE_FOR_NAME[name] = (
        dve_ops._CUSTOM_DVE_ROW_BASE + len(dve_ops.OPS) - 1
    )
    return op


def _register_const(nc, val, dtype=F32):
    key = (dtype, float(val))
    if key in nc.const_aps.aps:
        return
    t = nc.alloc_sbuf_tensor(f"const-{dtype.name}-{val}", [128, 1], dtype)
    nc.gpsimd.memset(t.ap(), float(val))
    nc.const_aps.aps[key] = t.ap()


def build(repeat=1, repeat_a=1):
    tail_op = _register_matern_tail()
    nc = bacc.Bacc(num_devices=1, debug=False)
    _register_const(nc, -LN3)
    _register_const(nc, 1.0)
    nc.all_engine_barrier()

    x = nc.dram_tensor("x", [ROWS, D], F32, kind="ExternalInput")
    y = nc.dram_tensor("y", [M, D], F32, kind="ExternalInput")
    W1 = nc.dram_tensor("W1", [L, D], F32, kind="ExternalInput")
    b1 = nc.dram_tensor("b1", [L], F32, kind="ExternalInput")
    W2 = nc.dram_tensor("W2", [1, L], F32, kind="ExternalInput")
    b2 = nc.dram_tensor("b2", [1], F32, kind="ExternalInput")
    out = nc.dram_tensor("out", [ROWS, M], F32, kind="ExternalOutput")

    with TileContext(nc) as tc:
        # persistent matmul-column tensors, live for the whole kernel
        with tc.tile_pool(name="keep", bufs=1) as kp:
            ycols = kp.tile([48, M], F32R)
            xcols = kp.tile([48, ROWS], F32R)
            for _ in range(repeat_a):
                _build_columns(nc, tc, x, y, W1, b1, W2, b2, ycols, xcols)
            for _ in range(repeat):
                _main_loop(nc, tc, out, ycols, xcols, tail_op)
    nc.compile()
    return nc


def _build_columns(nc, tc, x, y, W1, b1, W2, b2, ycols, xcols):
    """Stage A: per-point scales + the 16 matmul columns.

    Point grid layout: y point j = 64p + k lives at packed tile [p, 3k:3k+3].
    PE transposes give coords-on-partitions tiles; the MLP runs on a K=7
    blockdiag (two point-halves + folded b1). Scales come back to the packed
    layout through one [72,128] PE transpose.

    MLP column c of pts7 (c = 128*kk + p within a 4096/512 block):
      y (c < 4096): half A point 64p+kk, half B point 64p+kk+32
      x (c >= 4096): half A point 8p+kk', half B point 8p+kk'+4
    """
    with tc.tile_pool(name="mlp", bufs=1) as mp, \
         tc.tile_pool(name="mlp_tmp", bufs=2) as mt, \
         tc.tile_pool(name="mlp_psum", bufs=1, space="PSUM") as mpp, \
         tc.tile_pool(name="mlp_tpsum", bufs=2, space="PSUM") as tpp:
        # ---- natural-layout loads (contiguous per partition) -------------
        ypk = mp.tile([128, M * D // 128], F32)           # [128, 192]
        nc.sync.dma_start(ypk[:, :], y[:, :].flatten().rearrange(
            "(p k) -> p k", p=128))
        xpk = mp.tile([128, ROWS * D // 128], F32)        # [128, 24]
        nc.gpsimd.dma_start(xpk[:, :], x[:, :].flatten().rearrange(
            "(p k) -> p k", p=128))
        w1n = mp.tile([L, D], F32)
        nc.sync.dma_start(w1n[:, :], W1[:, :])
        w2n = mp.tile([1, L], F32)
        nc.gpsimd.dma_start(w2n[:, :], W2[:, :])
        b2s = mp.tile([1, 1], F32)
        nc.gpsimd.dma_start(
            b2s[:, :], b2[:].rearrange("(o one) -> o one", one=1))

        # identity for PE transposes
        ones128 = mp.tile([128, 128], F32)
        nc.vector.memset(ones128[:, :], 1.0)
        ident = mp.tile([128, 128], F32)
        nc.gpsimd.affine_select(
            ident[:, :], ones128[:, :], pattern=[[1, 128]],
            compare_op=alu.is_equal, fill=0.0, base=0, channel_multiplier=-1)

        # ---- coords-on-partitions via PE transposes ----------------------
        # one shared [128,128] PSUM tile tag for every transpose, sliced
        tp0 = tpp.tile([128, 128], F32, tag="tp")
        nc.tensor.transpose(tp0[0:96, :], ypk[:, 0:96], ident[:, :])
        ta = mp.tile([96, 128], F16)
        nc.vector.tensor_copy(ta[:, :], tp0[0:96, :])
        tp1 = tpp.tile([128, 128], F32, tag="tp")
        nc.tensor.transpose(tp1[0:96, :], ypk[:, 96:192], ident[:, :])
        tb = mp.tile([96, 128], F16)
        nc.vector.tensor_copy(tb[:, :], tp1[0:96, :])
        tp2 = tpp.tile([128, 128], F32, tag="tp")
        nc.tensor.transpose(tp2[0:24, :], xpk[:, :], ident[:, :])
        tx = mp.tile([24, 128], F16)
        nc.vector.tensor_copy(tx[:, :], tp2[0:24, :])

        # W1^T [3, 64] and the blockdiag-with-bias lhsT [7, 128]
        tp3 = tpp.tile([128, 128], F32, tag="tp")
        nc.tensor.transpose(tp3[0:D, 0:L], w1n[:, :], ident[0:L, 0:L])
        w1s = mp.tile([D, L], F16)
        nc.vector.tensor_copy(w1s[:, :], tp3[0:D, 0:L])
        w1t7 = mp.tile([D * 2 + 1, 128], F16)
        nc.vector.memset(w1t7[:, :], 0.0)
        nc.vector.tensor_copy(w1t7[0:D, 0:L], w1s[:, :])
        nc.gpsimd.dma_start(w1t7[D:2 * D, L:128], w1s[:, :])
        nc.gpsimd.dma_start(
            w1t7[2 * D:2 * D + 1, 0:L], b1[:].rearrange("(one l) -> one l", one=1))
        nc.gpsimd.dma_start(
            w1t7[2 * D:2 * D + 1, L:128], b1[:].rearrange("(one l) -> one l", one=1))

        # pts7 [7, HALF]: partitions 0-2 half A coords, 3-5 half B, 6 ones
        # (memset whole tile; rows 0-5 are fully overwritten by the DMAs)
        pts7 = mp.tile([D * 2 + 1, HALF], F16)
        nc.vector.memset(pts7[:, :], 1.0)
        for d in range(D):
            nc.sync.dma_start(pts7[d:d + 1, 0:M // 2], ta[d::D, :])
            nc.sync.dma_start(pts7[D + d:D + d + 1, 0:M // 2], tb[d::D, :])
            nc.gpsimd.dma_start(
                pts7[d:d + 1, M // 2:HALF], tx[d:12:D, :])
            nc.gpsimd.dma_start(
                pts7[D + d:D + d + 1, M // 2:HALF], tx[12 + d::D, :])

        # W2 scaled by -selu_lambda (folded so hsel can be alpha*t - r)
        tp4 = tpp.tile([128, 128], F32, tag="tp")
        nc.tensor.transpose(tp4[0:L, 0:1], w2n[:, :], ident[0:1, 0:1])
        w2s = mp.tile([L, 1], F16)
        nc.vector.tensor_scalar_mul(w2s[:, :], tp4[0:L, 0:1], -SELU_L)
        w2stack = mp.tile([128, 2], F16)
        nc.vector.memset(w2stack[:, :], 0.0)
        nc.vector.tensor_copy(w2stack[0:L, 0:1], w2s[:, :])
        nc.vector.tensor_copy(w2stack[L:128, 1:2], w2s[:, :])

        # b2 broadcast [2, 1] via ones-matmul (partition replication)
        ones2 = mp.tile([1, 2], F32)
        nc.vector.memset(ones2[:, :], 1.0)
        tp5 = tpp.tile([128, 128], F32, tag="tp")
        nc.tensor.matmul(tp5[0:2, 0:1], lhsT=ones2[:, :], rhs=b2s[:, :],
                         start=True, stop=True)
        b2b = mp.tile([2, 1], F32)
        nc.vector.tensor_copy(b2b[:, :], tp5[0:2, 0:1])

        # ---- hidden layer + selu pieces ---------------------------------
        #   e = exp(z); t = relu(1-e) [in place]; r = relu(z)
        #   hsel = alpha*t - r   (= -selu(z)/lambda; lambda folded in W2)
        hsel = mp.tile([128, HALF], F16)
        for c0 in range(0, HALF, GRP):
            cw = min(GRP, HALF - c0)
            ph = mpp.tile([128, GRP], F32, tag="ph")
            for j in range(0, cw, MMW):
                nc.tensor.matmul(
                    ph[:, j:j + MMW],
                    lhsT=w1t7[:, :],
                    rhs=pts7[:, c0 + j:c0 + j + MMW],
                    start=True, stop=True,
                )
            ec = mt.tile([128, GRP], F32, tag="ec")
            nc.scalar.activation(ec[:, 0:cw], ph[:, 0:cw], Act.Exp)
            nc.scalar.activation(
                ec[:, 0:cw], ec[:, 0:cw], Act.Relu, bias=1.0, scale=-1.0)
            rc = mt.tile([128, GRP], F32, tag="rc")
            nc.vector.tensor_scalar(
                rc[:, 0:cw], ph[:, 0:cw], 0.0, 0.0,
                op0=alu.add, op1=alu.max,
            )
            nc.vector.scalar_tensor_tensor(
                hsel[:, c0:c0 + cw], ec[:, 0:cw], SELU_A, rc[:, 0:cw],
                op0=alu.mult, op1=alu.subtract,
            )

        # ---- output layer: z rows [2, HALF]; softplus straight off PSUM --
        # softplus(z+b2) = ln(1+e^(z+b2)): exp while 2-lane, ln after the
        # regroup once the data is 72 partitions wide
        sp = mp.tile([2, HALF], F32)
        for c0 in range(0, HALF, MMW):
            pz = mpp.tile([2, MMW], F32, tag="pz")
            nc.tensor.matmul(
                pz[:, :],
                lhsT=w2stack[:, :],
                rhs=hsel[:, c0:c0 + MMW],
                start=True, stop=True,
            )
            nc.scalar.activation(
                sp[:, c0:c0 + MMW], pz[:, :], Act.Exp, bias=b2b[:, :])

        # ---- scales back to packed layout via one [72,128] transpose ----
        # pzg row jj: y k=jj (jj<64), x k=jj-64 (jj>=64)
        pzg = mp.tile([YG + XG, 128], F32)
        nc.sync.dma_start(pzg[0:32, :], sp[0:1, 0:M // 2])
        nc.sync.dma_start(pzg[32:64, :], sp[1:2, 0:M // 2])
        nc.sync.dma_start(pzg[64:68, :], sp[0:1, M // 2:HALF])
        nc.sync.dma_start(pzg[68:72, :], sp[1:2, M // 2:HALF])
        nc.scalar.activation(pzg[:, :], pzg[:, :], Act.Ln, bias=1.0)
        tp6 = tpp.tile([128, 128], F32, tag="tp")
        nc.tensor.transpose(
            tp6[:, 0:YG + XG], pzg[:, :], ident[0:YG + XG, 0:YG + XG])
        syq = mp.tile([128, YG + XG], F32)
        nc.vector.tensor_copy(syq[:, :], tp6[:, 0:YG + XG])
        syp = syq[:, 0:YG]                    # [128, 64] s(y[64p+k])
        sxp = syq[:, YG:YG + XG]              # [128, 8]  s(x[8p+k])
        sy2p = mp.tile([128, YG], F32)
        nc.vector.tensor_mul(sy2p[:, :], syp, syp)
        sx2p = mp.tile([128, XG], F32)
        nc.vector.tensor_mul(sx2p[:, :], sxp, sxp)

        # ---- packed |p|^2 ------------------------------------------------
        def norms(src, npts, tag):
            k = npts // 128
            t0 = mp.tile([128, k], F32, tag=tag)
            t1 = mp.tile([128, k], F32, tag=tag + "b")
            nc.vector.tensor_mul(t0[:, :], src[:, 0::D], src[:, 0::D])
            nc.vector.tensor_mul(t1[:, :], src[:, 1::D], src[:, 1::D])
            nc.vector.tensor_add(t0[:, :], t0[:, :], t1[:, :])
            nc.vector.tensor_mul(t1[:, :], src[:, 2::D], src[:, 2::D])
            nc.vector.tensor_add(t0[:, :], t0[:, :], t1[:, :])
            return t0

        n2yp = norms(ypk, M, "nrmy")      # [128, 64], point 64p+k at [p, k]
        n2xp = norms(xpk, ROWS, "nrmx")   # [128, 8]

        onesy = mp.tile([128, YG], F32)
        nc.vector.memset(onesy[:, :], 1.0)

        # ---- build the 16 matmul columns --------------------------------
        # w~ = sum_p xcol[p](i) * ycol[p](j) = 5*r2*S^2 + CLAMP_EPS
        # p = 3a+b (a<5, b<3), p=15 the clamp column.
        # x side: f_a in {n2x, 1, x0, x1, x2}, h_b in {sx^2, sx, 1},
        #         coeff ca*cb folded into the x side
        # y side: g_a in {1, n2y, y0, y1, y2}, k_b in {1, sy, sy^2}
        # Products are computed in the packed [128, pts/128] layout, staged
        # to DRAM rows (partition-parallel both ways), then loaded as the
        # [16, pts] matmul operand per column quarter.
        sfx = nc.next_id()
        ych_stage = nc.dram_tensor(f"ych_stage{sfx}", [16, M], F32R)
        yclo_stage = nc.dram_tensor(f"yclo_stage{sfx}", [16, M], F32R)
        xch_stage = nc.dram_tensor(f"xch_stage{sfx}", [16, ROWS], F32R)
        xclo_stage = nc.dram_tensor(f"xclo_stage{sfx}", [16, ROWS], F32R)
        ca = [5.0, 5.0, -10.0, -10.0, -10.0]
        cb = [1.0, 2.0, 1.0]
        gy = [onesy, n2yp, ypk[:, 0::D], ypk[:, 1::D], ypk[:, 2::D]]
        ky = [None, syp, sy2p]
        fx = [n2xp, None, xpk[:, 0::D], xpk[:, 1::D], xpk[:, 2::D]]
        hx = [sx2p, sxp, None]
        # y-side products first (they gate the main loop), kept alive
        # so staging can go out in column chunks
        prybig = mp.tile([128, 16 * YG], F32)
        for a in range(5):
            for b in range(3):
                p = 3 * a + b
                psl = slice(p * YG, (p + 1) * YG)
                ga, kb = gy[a], ky[b]
                if kb is None:
                    nc.vector.tensor_copy(prybig[:, psl], ga)
                else:
                    nc.vector.tensor_mul(prybig[:, psl], ga, kb[:, :])
        nc.vector.memset(prybig[:, 15 * YG:16 * YG], 1.0)
        # compensated split in the packed layout (full-lane DVE)
        pryh = mp.tile([128, 16 * YG], F32R)
        nc.gpsimd.dma_start(pryh[:, :], prybig[:, :])
        prylf = mp.tile([128, 16 * YG], F32)
        nc.vector.tensor_sub(prylf[:, :], prybig[:, :], pryh[:, :].bitcast(F32))
        prylo = mp.tile([128, 16 * YG], F32R)
        nc.gpsimd.dma_start(prylo[:, :], prylf[:, :])
        # stage + load by column quarter so the first matmuls can start
        # before the whole column tensor is assembled
        # Column rows (compensated f32r, K=48):
        #   [0:16)  x_hi | y_hi, [16:32) x_hi | y_lo, [32:48) x_lo | y_hi
        # so w = x.y_hi + x_hi.y_lo + x_lo.y_hi: the f32r rounding residue
        # cancels to ~(2^-m)^2 and near-zero distances stay accurate.
        QC = M // 4
        PQ = QC // YG                  # packed partitions per quarter
        for ci in range(4):
            qsl = slice(ci * QC, (ci + 1) * QC)
            prt = slice(ci * PQ, (ci + 1) * PQ)
            # one issue per (quarter, hi/lo): the DRAM AP presents the
            # packed (q, p, k) order so the SBUF side streams linearly
            nc.sync.dma_start(
                ych_stage[:, qsl].rearrange("p (q k) -> q p k", k=YG),
                pryh[prt, :])
            nc.gpsimd.dma_start(
                yclo_stage[:, qsl].rearrange("p (q k) -> q p k", k=YG),
                prylo[prt, :])
            nc.scalar.dma_start(ycols[0:16, qsl], ych_stage[:, qsl])
            nc.scalar.dma_start(ycols[16:32, qsl], yclo_stage[:, qsl])
            nc.scalar.dma_start(ycols[32:48, qsl], ych_stage[:, qsl])

        # x side (small); row 15 = (CLAMP_EPS on x) * (1 on y)
        prxbig = mp.tile([128, 16 * XG], F32)
        for a in range(5):
            for b in range(3):
                p = 3 * a + b
                psl = slice(p * XG, (p + 1) * XG)
                coeff = ca[a] * cb[b]
                fa, hb = fx[a], hx[b]
                if fa is None and hb is None:
                    nc.vector.memset(prxbig[:, psl], coeff)
                elif fa is None:
                    nc.vector.tensor_scalar_mul(prxbig[:, psl], hb, coeff)
                elif hb is None:
                    nc.vector.tensor_scalar_mul(prxbig[:, psl], fa, coeff)
                else:
                    nc.vector.scalar_tensor_tensor(
                        prxbig[:, psl], fa, coeff, hb,
                        op0=alu.mult, op1=alu.mult)
        nc.vector.memset(prxbig[:, 15 * XG:16 * XG], CLAMP_EPS)
        prxh = mp.tile([128, 16 * XG], F32R)
        nc.gpsimd.dma_start(prxh[:, :], prxbig[:, :])
        prxlf = mp.tile([128, 16 * XG], F32)
        nc.vector.tensor_sub(prxlf[:, :], prxbig[:, :], prxh[:, :].bitcast(F32))
        prxlo = mp.tile([128, 16 * XG], F32R)
        nc.gpsimd.dma_start(prxlo[:, :], prxlf[:, :])
        nc.sync.dma_start(
            xch_stage[:, :].rearrange("p (q k) -> q p k", k=XG), prxh[:, :])
        nc.gpsimd.dma_start(
            xclo_stage[:, :].rearrange("p (q k) -> q p k", k=XG), prxlo[:, :])
        nc.scalar.dma_start(xcols[0:16, :], xch_stage[:, :])
        nc.scalar.dma_start(xcols[16:32, :], xch_stage[:, :])
        nc.scalar.dma_start(xcols[32:48, :], xclo_stage[:, :])


def _main_loop(nc, tc, out, ycols, xcols, tail_op):
    # Per phase (4 strips of 128 rows):
    #   [sqrt table]  per strip, per 2048-col group: 4 f32r K=16 matmuls
    #                 -> PSUM, then ACT sqrt -> strip-wide u tile (fp16)
    #   [exp table]   per strip: one strip-wide e3 = exp(-u - ln3) (fp16)
    #   DVE tail + output DMA per 2048-col group
    # Batching all 16 sqrts before the 4 exps keeps it to 2 ACT table
    # switches per phase (sqrt and exp live in different table sets).
    with tc.tile_pool(name="main_psum", bufs=2, space="PSUM") as pp, \
         tc.tile_pool(name="upool", bufs=PHASE_STRIPS + 1) as up, \
         tc.tile_pool(name="epool", bufs=PHASE_STRIPS) as ep, \
         tc.tile_pool(name="wpool", bufs=4) as wp, \
         tc.tile_pool(name="opool", bufs=3) as op_:
        ph0 = 0
        for nph in PHASE_SIZES:
            strips = range(ph0, ph0 + nph)
            ph0 += nph
            utiles = {}
            for s in strips:
                lhs = xcols[:, s * 128:(s + 1) * 128]
                u = up.tile([128, M], F16, tag="u")
                utiles[s] = u
                for g in range(N_GRP):
                    pw = pp.tile([128, GRP], F32, tag="pw")
                    for j in range(0, GRP, MMW):
                        nc.tensor.matmul(
                            pw[:, j:j + MMW],
                            lhsT=lhs,
                            rhs=ycols[:, g * GRP + j:g * GRP + j + MMW],
                            start=True, stop=True,
                        )
                    # DVE drains PSUM ~2x faster than sqrt would, so the
                    # matmuls can run ahead into the wpool instead of
                    # stalling on the 2-group PSUM capacity
                    wt = wp.tile([128, GRP], F32, tag="wt")
                    nc.vector.tensor_copy(wt[:, :], pw[:, :])
                    nc.scalar.activation(
                        u[:, g * GRP:(g + 1) * GRP], wt[:, :], Act.Sqrt)
            for s in strips:
                e3 = ep.tile([128, M], F16, tag="e3")
                if EXP_STRIPWISE:
                    nc.scalar.activation(
                        e3[:, :], utiles[s][:, :], Act.Exp,
                        bias=-LN3, scale=-1.0)
                for g in range(N_GRP):
                    sl = slice(g * GRP, (g + 1) * GRP)
                    if not EXP_STRIPWISE:
                        nc.scalar.activation(
                            e3[:, sl], utiles[s][:, sl], Act.Exp,
                            bias=-LN3, scale=-1.0)
                    o = op_.tile([128, GRP], F32, tag="o")
                    nc.vector._custom_dve(
                        tail_op, out=o[:, :], in0=utiles[s][:, sl],
                        in1=e3[:, sl], s0=1.5, s1=0.75,
                    )
                    nc.sync.dma_start(
                        out[s * 128:(s + 1) * 128, g * GRP:(g + 1) * GRP],
                        o[:, :],
                    )


_NC_CACHE = None


def kernel(**inputs):
    global _NC_CACHE
    if _NC_CACHE is None:
        _NC_CACHE = build()
    nc = _NC_CACHE
    x = np.ascontiguousarray(np.asarray(inputs["x"], dtype=np.float32))
    base = {
        "y": np.ascontiguousarray(np.asarray(inputs["y"], dtype=np.float32)),
        "W1": np.ascontiguousarray(np.asarray(inputs["W1"], dtype=np.float32)),
        "b1": np.ascontiguousarray(np.asarray(inputs["b1"], dtype=np.float32)),
        "W2": np.ascontiguousarray(np.asarray(inputs["W2"], dtype=np.float32)),
        "b2": np.ascontiguousarray(np.asarray(inputs["b2"], dtype=np.float32)),
    }
    in_maps = [
        {"x": x[c * ROWS:(c + 1) * ROWS], **base} for c in range(N_CORES)
    ]
    res = run_bass_kernel_spmd(nc, in_maps, core_ids=list(range(N_CORES)))
    return np.concatenate([res.results[c]["out"] for c in range(N_CORES)], axis=0)


# revision 23
# speedup vs baseline: 1.0730x; 1.0730x over previous
"""Nonstationary Matern-5/2 kernel matrix on 8 Trainium2 NeuronCores.

Math: out[i,j] = (1 + u + u^2/3) * exp(-u),  u = sqrt5 * r_ij * (s(x_i)+s(y_j))
where r_ij = ||x_i - y_j|| and s() is a tiny MLP (Linear-selu-Linear-softplus).

Key trick: u^2 = 5*r2*S^2 where r2 (rank-5 in outer-product terms) and
S^2 = (sx+sy)^2 (rank-3) multiply elementwise into a rank-15 sum, so ONE
K=16 matmul per output tile produces w = u^2 (+ a constant clamp column).
Then per tile: u = sqrt(w) on ACT, e = exp(-u - ln3) on ACT, and
out = ((u+1.5)^2 + 0.75) * e in a single fused custom DVE op
(since 1+u+u^2/3 = ((u+1.5)^2 + 0.75)/3).

Perf structure (v2):
 - All x/y point handling starts from the packed natural layout
   ([128, 3k] contiguous per partition); coords-on-partitions layouts are
   derived via PE-array transposes, never via 4-byte-granular DMA.
 - Matmuls run as float32r (full PE rate at moving width 512, ~4x fp32).
 - Main loop phases of 4 strips batch all sqrts before all exps so the
   ACT table set switches only twice per phase.
 - exp output is fp16 (halves its SBUF/bandwidth), final out fp32.

Sharding: data-parallel over rows of x; each core computes a [1024, 8192]
block; y and MLP params replicated.
"""

import numpy as np

import concourse.bacc as bacc
import concourse.bass as bass
import concourse.mybir as mybir
from concourse.mybir import AluOpType as alu
from concourse.tile import TileContext
from concourse.bass_utils import run_bass_kernel_spmd

import concourse.dve_ops as dve_ops
from concourse.dve_spec import Spec, Src0, Src1, C0, C1, sq, lower
from concourse.dve_uop import DveOpSpec

N, M, D, L = 8192, 8192, 3, 64
N_CORES = 8
ROWS = N // N_CORES          # 1024 x-rows per core
N_STRIPS = ROWS // 128       # 8 strips of 128 partitions
GRP = 2048                   # supertile free width
N_GRP = M // GRP             # 4 col groups per strip
MMW = 512                    # matmul moving width (>=256 keeps f32r at rate)
PHASE_STRIPS = 3             # max strips per sqrt/exp table phase
PHASE_SIZES = [3, 3, 1, 1]   # shrinking phases so the output DMA tail overlaps
EXP_STRIPWISE = True         # one exp instr per strip (coarse deps beat thrash)

NPTS = ROWS + M              # 9216 points: x-shard + y
HALF = NPTS // 2             # 4608 columns per blockdiag half
YG = M // 128                # 64 y grid k-rows
XG = ROWS // 128             # 8 x grid k-rows

LN3 = float(np.log(3.0))
SELU_L = 1.0507009873554805
SELU_A = 1.6732632423543772
CLAMP_EPS = 5e-3             # w = u^2 clamp floor (vs matmul noise)

F32 = mybir.dt.float32
F16 = mybir.dt.float16
F32R = mybir.dt.float32r
Act = mybir.ActivationFunctionType


def _register_matern_tail():
    """out = ((in0 + s0)^2 + s1) * in1, one fused DVE instruction."""
    name = "MATERN_TAIL_ANT"
    for o in dve_ops.OPS:
        if o.name == name:
            return o
    spec = Spec(
        body=(sq(Src0 + C0) + C1) * Src1,
        reference=lambda in0, in1, s0, s1, imm2: (
            ((in0.astype(np.float32) + s0) ** 2 + s1) * in1.astype(np.float32)
        ).astype(np.float32),
    )
    shas = {}
    for ver in ("v3", "v4"):
        uops = lower(spec, ver=ver)
        shas[ver] = DveOpSpec(name=name, opcode=1, uops=uops, rd1_en=True).sha(ver)
    op = dve_ops.DveOp(name, spec, subdim=False, uops_sha=shas)
    dve_ops.OPS.append(op)
    dve_ops.CUSTOM_DVE_SPECS[name] = spec
    dve_ops._SUB_OPCODE_FOR_NAME[name] = (
        dve_ops._CUSTOM_DVE_ROW_BASE + len(dve_ops.OPS) - 1
    )
    return op


def _register_const(nc, val, dtype=F32):
    key = (dtype, float(val))
    if key in nc.const_aps.aps:
        return
    t = nc.alloc_sbuf_tensor(f"const-{dtype.name}-{val}", [128, 1], dtype)
    nc.gpsimd.memset(t.ap(), float(val))
    nc.const_aps.aps[key] = t.ap()


def build(repeat=1, repeat_a=1):
    tail_op = _register_matern_tail()
    nc = bacc.Bacc(num_devices=1, debug=False)
    _register_const(nc, -LN3)
    _register_const(nc, 1.0)
    nc.all_engine_barrier()

    x = nc.dram_tensor("x", [ROWS, D], F32, kind="ExternalInput")
    y = nc.dram_tensor("y", [M, D], F32, kind="ExternalInput")
    W1 = nc.dram_tensor("W1", [L, D], F32, kind="ExternalInput")
    b1 = nc.dram_tensor("b1", [L], F32, kind="ExternalInput")
    W2 = nc.dram_tensor("W2", [1, L], F32, kind="ExternalInput")
    b2 = nc.dram_tensor("b2", [1], F32, kind="ExternalInput")
    out = nc.dram_tensor("out", [ROWS, M], F32, kind="ExternalOutput")

    with TileContext(nc) as tc:
        # persistent matmul-column tensors, live for the whole kernel
        with tc.tile_pool(name="keep", bufs=1) as kp:
            ycols = kp.tile([48, M], F32R)
            xcols = kp.tile([48, ROWS], F32R)
            for _ in range(repeat_a):
                _build_columns(nc, tc, x, y, W1, b1, W2, b2, ycols, xcols)
            for _ in range(repeat):
                _main_loop(nc, tc, out, ycols, xcols, tail_op)
    nc.compile()
    return nc


def _build_columns(nc, tc, x, y, W1, b1, W2, b2, ycols, xcols):
    """Stage A: per-point scales + the 16 matmul columns.

    Point grid layout: y point j = 64p + k lives at packed tile [p, 3k:3k+3].
    PE transposes give coords-on-partitions tiles; the MLP runs on a K=7
    blockdiag (two point-halves + folded b1). Scales come back to the packed
    layout through one [72,128] PE transpose.

    MLP column c of pts7 (c = 128*kk + p within a 4096/512 block):
      y (c < 4096): half A point 64p+kk, half B point 64p+kk+32
      x (c >= 4096): half A point 8p+kk', half B point 8p+kk'+4
    """
    with tc.tile_pool(name="mlp", bufs=1) as mp, \
         tc.tile_pool(name="mlp_tmp", bufs=2) as mt, \
         tc.tile_pool(name="mlp_psum", bufs=1, space="PSUM") as mpp, \
         tc.tile_pool(name="mlp_tpsum", bufs=2, space="PSUM") as tpp:
        # ---- natural-layout loads (contiguous per partition) -------------
        ypk = mp.tile([128, M * D // 128], F32)           # [128, 192]
        nc.sync.dma_start(ypk[:, :], y[:, :].flatten().rearrange(
            "(p k) -> p k", p=128))
        xpk = mp.tile([128, ROWS * D // 128], F32)        # [128, 24]
        nc.gpsimd.dma_start(xpk[:, :], x[:, :].flatten().rearrange(
            "(p k) -> p k", p=128))
        w1n = mp.tile([L, D], F32)
        nc.sync.dma_start(w1n[:, :], W1[:, :])
        w2n = mp.tile([1, L], F32)
        nc.gpsimd.dma_start(w2n[:, :], W2[:, :])
        b2s = mp.tile([1, 1], F32)
        nc.gpsimd.dma_start(
            b2s[:, :], b2[:].rearrange("(o one) -> o one", one=1))

        # identity for PE transposes
        ones128 = mp.tile([128, 128], F32)
        nc.vector.memset(ones128[:, :], 1.0)
        ident = mp.tile([128, 128], F32)
        nc.gpsimd.affine_select(
            ident[:, :], ones128[:, :], pattern=[[1, 128]],
            compare_op=alu.is_equal, fill=0.0, base=0, channel_multiplier=-1)

        # ---- coords-on-partitions via PE transposes ----------------------
        # one shared [128,128] PSUM tile tag for every transpose, sliced
        tp0 = tpp.tile([128, 128], F32, tag="tp")
        nc.tensor.transpose(tp0[0:96, :], ypk[:, 0:96], ident[:, :])
        ta = mp.tile([96, 128], F16)
        nc.vector.tensor_copy(ta[:, :], tp0[0:96, :])
        tp1 = tpp.tile([128, 128], F32, tag="tp")
        nc.tensor.transpose(tp1[0:96, :], ypk[:, 96:192], ident[:, :])
        tb = mp.tile([96, 128], F16)
        nc.vector.tensor_copy(tb[:, :], tp1[0:96, :])
        tp2 = tpp.tile([128, 128], F32, tag="tp")
        nc.tensor.transpose(tp2[0:24, :], xpk[:, :], ident[:, :])
        tx = mp.tile([24, 128], F16)
        nc.vector.tensor_copy(tx[:, :], tp2[0:24, :])

        # W1^T [3, 64] and the blockdiag-with-bias lhsT [7, 128]
        tp3 = tpp.tile([128, 128], F32, tag="tp")
        nc.tensor.transpose(tp3[0:D, 0:L], w1n[:, :], ident[0:L, 0:L])
        w1s = mp.tile([D, L], F16)
        nc.vector.tensor_copy(w1s[:, :], tp3[0:D, 0:L])
        w1t7 = mp.tile([D * 2 + 1, 128], F16)
        nc.vector.memset(w1t7[:, :], 0.0)
        nc.vector.tensor_copy(w1t7[0:D, 0:L], w1s[:, :])
        nc.gpsimd.dma_start(w1t7[D:2 * D, L:128], w1s[:, :])
        nc.gpsimd.dma_start(
            w1t7[2 * D:2 * D + 1, 0:L], b1[:].rearrange("(one l) -> one l", one=1))
        nc.gpsimd.dma_start(
            w1t7[2 * D:2 * D + 1, L:128], b1[:].rearrange("(one l) -> one l", one=1))

        # pts7 [7, HALF]: partitions 0-2 half A coords, 3-5 half B, 6 ones
        # (rows 0-5 fully written by the regroup DMAs; row 6 DMA'd from a
        # small ones tile -- a full-tile memset would cost ~4us single-lane)
        pts7 = mp.tile([D * 2 + 1, HALF], F16)
        ones_src = mp.tile([32, HALF // 32], F16)
        nc.vector.memset(ones_src[:, :], 1.0)
        nc.sync.dma_start(pts7[2 * D:2 * D + 1, :], ones_src[:, :])
        for d in range(D):
            nc.sync.dma_start(pts7[d:d + 1, 0:M // 2], ta[d::D, :])
            nc.sync.dma_start(pts7[D + d:D + d + 1, 0:M // 2], tb[d::D, :])
            nc.gpsimd.dma_start(
                pts7[d:d + 1, M // 2:HALF], tx[d:12:D, :])
            nc.gpsimd.dma_start(
                pts7[D + d:D + d + 1, M // 2:HALF], tx[12 + d::D, :])

        # W2 scaled by -selu_lambda (folded so hsel can be alpha*t - r)
        tp4 = tpp.tile([128, 128], F32, tag="tp")
        nc.tensor.transpose(tp4[0:L, 0:1], w2n[:, :], ident[0:1, 0:1])
        w2s = mp.tile([L, 1], F16)
        nc.vector.tensor_scalar_mul(w2s[:, :], tp4[0:L, 0:1], -SELU_L)
        w2stack = mp.tile([128, 2], F16)
        nc.vector.memset(w2stack[:, :], 0.0)
        nc.vector.tensor_copy(w2stack[0:L, 0:1], w2s[:, :])
        nc.vector.tensor_copy(w2stack[L:128, 1:2], w2s[:, :])

        # b2 broadcast [2, 1] via ones-matmul (partition replication)
        ones2 = mp.tile([1, 2], F32)
        nc.vector.memset(ones2[:, :], 1.0)
        tp5 = tpp.tile([128, 128], F32, tag="tp")
        nc.tensor.matmul(tp5[0:2, 0:1], lhsT=ones2[:, :], rhs=b2s[:, :],
                         start=True, stop=True)
        b2b = mp.tile([2, 1], F32)
        nc.vector.tensor_copy(b2b[:, :], tp5[0:2, 0:1])

        # ---- hidden layer + selu pieces ---------------------------------
        #   e = exp(z); t = relu(1-e) [in place]; r = relu(z)
        #   hsel = alpha*t - r   (= -selu(z)/lambda; lambda folded in W2)
        hsel = mp.tile([128, HALF], F16)
        for c0 in range(0, HALF, GRP):
            cw = min(GRP, HALF - c0)
            ph = mpp.tile([128, GRP], F32, tag="ph")
            for j in range(0, cw, MMW):
                nc.tensor.matmul(
                    ph[:, j:j + MMW],
                    lhsT=w1t7[:, :],
                    rhs=pts7[:, c0 + j:c0 + j + MMW],
                    start=True, stop=True,
                )
            ec = mt.tile([128, GRP], F32, tag="ec")
            nc.scalar.activation(ec[:, 0:cw], ph[:, 0:cw], Act.Exp)
            nc.scalar.activation(
                ec[:, 0:cw], ec[:, 0:cw], Act.Relu, bias=1.0, scale=-1.0)
            rc = mt.tile([128, GRP], F32, tag="rc")
            nc.vector.tensor_scalar(
                rc[:, 0:cw], ph[:, 0:cw], 0.0, 0.0,
                op0=alu.add, op1=alu.max,
            )
            nc.vector.scalar_tensor_tensor(
                hsel[:, c0:c0 + cw], ec[:, 0:cw], SELU_A, rc[:, 0:cw],
                op0=alu.mult, op1=alu.subtract,
            )

        # ---- output layer: z rows [2, HALF]; softplus straight off PSUM --
        # softplus(z+b2) = ln(1+e^(z+b2)): exp while 2-lane, ln after the
        # regroup once the data is 72 partitions wide
        sp = mp.tile([2, HALF], F32)
        for c0 in range(0, HALF, MMW):
            pz = mpp.tile([2, MMW], F32, tag="pz")
            nc.tensor.matmul(
                pz[:, :],
                lhsT=w2stack[:, :],
                rhs=hsel[:, c0:c0 + MMW],
                start=True, stop=True,
            )
            nc.scalar.activation(
                sp[:, c0:c0 + MMW], pz[:, :], Act.Exp, bias=b2b[:, :])

        # ---- scales back to packed layout via one [72,128] transpose ----
        # pzg row jj: y k=jj (jj<64), x k=jj-64 (jj>=64)
        pzg = mp.tile([YG + XG, 128], F32)
        nc.sync.dma_start(pzg[0:32, :], sp[0:1, 0:M // 2])
        nc.sync.dma_start(pzg[32:64, :], sp[1:2, 0:M // 2])
        nc.sync.dma_start(pzg[64:68, :], sp[0:1, M // 2:HALF])
        nc.sync.dma_start(pzg[68:72, :], sp[1:2, M // 2:HALF])
        nc.scalar.activation(pzg[:, :], pzg[:, :], Act.Ln, bias=1.0)
        # touch sqrt so its table set loads now, while ACT is otherwise idle
        sqrt_warm = mp.tile([32, 1], F32)
        nc.scalar.activation(sqrt_warm[:, :], ident[0:32, 0:1], Act.Sqrt)
        tp6 = tpp.tile([128, 128], F32, tag="tp")
        nc.tensor.transpose(
            tp6[:, 0:YG + XG], pzg[:, :], ident[0:YG + XG, 0:YG + XG])
        syq = mp.tile([128, YG + XG], F32)
        nc.vector.tensor_copy(syq[:, :], tp6[:, 0:YG + XG])
        syp = syq[:, 0:YG]                    # [128, 64] s(y[64p+k])
        sxp = syq[:, YG:YG + XG]              # [128, 8]  s(x[8p+k])
        sy2p = mp.tile([128, YG], F32)
        nc.vector.tensor_mul(sy2p[:, :], syp, syp)
        sx2p = mp.tile([128, XG], F32)
        nc.vector.tensor_mul(sx2p[:, :], sxp, sxp)

        # ---- packed |p|^2 ------------------------------------------------
        def norms(src, npts, tag):
            k = npts // 128
            t0 = mp.tile([128, k], F32, tag=tag)
            t1 = mp.tile([128, k], F32, tag=tag + "b")
            nc.vector.tensor_mul(t0[:, :], src[:, 0::D], src[:, 0::D])
            nc.vector.tensor_mul(t1[:, :], src[:, 1::D], src[:, 1::D])
            nc.vector.tensor_add(t0[:, :], t0[:, :], t1[:, :])
            nc.vector.tensor_mul(t1[:, :], src[:, 2::D], src[:, 2::D])
            nc.vector.tensor_add(t0[:, :], t0[:, :], t1[:, :])
            return t0

        n2yp = norms(ypk, M, "nrmy")      # [128, 64], point 64p+k at [p, k]
        n2xp = norms(xpk, ROWS, "nrmx")   # [128, 8]

        onesy = mp.tile([128, YG], F32)
        nc.vector.memset(onesy[:, :], 1.0)

        # ---- build the 16 matmul columns --------------------------------
        # w~ = sum_p xcol[p](i) * ycol[p](j) = 5*r2*S^2 + CLAMP_EPS
        # p = 3a+b (a<5, b<3), p=15 the clamp column.
        # x side: f_a in {n2x, 1, x0, x1, x2}, h_b in {sx^2, sx, 1},
        #         coeff ca*cb folded into the x side
        # y side: g_a in {1, n2y, y0, y1, y2}, k_b in {1, sy, sy^2}
        # Products are computed in the packed [128, pts/128] layout, staged
        # to DRAM rows (partition-parallel both ways), then loaded as the
        # [16, pts] matmul operand per column quarter.
        sfx = nc.next_id()
        ych_stage = nc.dram_tensor(f"ych_stage{sfx}", [16, M], F32R)
        yclo_stage = nc.dram_tensor(f"yclo_stage{sfx}", [16, M], F32R)
        xch_stage = nc.dram_tensor(f"xch_stage{sfx}", [16, ROWS], F32R)
        xclo_stage = nc.dram_tensor(f"xclo_stage{sfx}", [16, ROWS], F32R)
        ca = [5.0, 5.0, -10.0, -10.0, -10.0]
        cb = [1.0, 2.0, 1.0]
        gy = [onesy, n2yp, ypk[:, 0::D], ypk[:, 1::D], ypk[:, 2::D]]
        ky = [None, syp, sy2p]
        fx = [n2xp, None, xpk[:, 0::D], xpk[:, 1::D], xpk[:, 2::D]]
        hx = [sx2p, sxp, None]
        # y-side products first (they gate the main loop), kept alive
        # so staging can go out in column chunks
        prybig = mp.tile([128, 16 * YG], F32)
        for a in range(5):
            for b in range(3):
                p = 3 * a + b
                psl = slice(p * YG, (p + 1) * YG)
                ga, kb = gy[a], ky[b]
                if kb is None:
                    nc.vector.tensor_copy(prybig[:, psl], ga)
                else:
                    nc.vector.tensor_mul(prybig[:, psl], ga, kb[:, :])
        nc.vector.memset(prybig[:, 15 * YG:16 * YG], 1.0)
        # compensated split in the packed layout (full-lane DVE)
        pryh = mp.tile([128, 16 * YG], F32R)
        nc.gpsimd.dma_start(pryh[:, :], prybig[:, :])
        prylf = mp.tile([128, 16 * YG], F32)
        nc.vector.tensor_sub(prylf[:, :], prybig[:, :], pryh[:, :].bitcast(F32))
        prylo = mp.tile([128, 16 * YG], F32R)
        nc.gpsimd.dma_start(prylo[:, :], prylf[:, :])
        # stage + load by column quarter so the first matmuls can start
        # before the whole column tensor is assembled
        # x side (small); row 15 = (CLAMP_EPS on x) * (1 on y)
        prxbig = mp.tile([128, 16 * XG], F32)
        for a in range(5):
            for b in range(3):
                p = 3 * a + b
                psl = slice(p * XG, (p + 1) * XG)
                coeff = ca[a] * cb[b]
                fa, hb = fx[a], hx[b]
                if fa is None and hb is None:
                    nc.vector.memset(prxbig[:, psl], coeff)
                elif fa is None:
                    nc.vector.tensor_scalar_mul(prxbig[:, psl], hb, coeff)
                elif hb is None:
                    nc.vector.tensor_scalar_mul(prxbig[:, psl], fa, coeff)
                else:
                    nc.vector.scalar_tensor_tensor(
                        prxbig[:, psl], fa, coeff, hb,
                        op0=alu.mult, op1=alu.mult)
        nc.vector.memset(prxbig[:, 15 * XG:16 * XG], CLAMP_EPS)
        prxh = mp.tile([128, 16 * XG], F32R)
        nc.gpsimd.dma_start(prxh[:, :], prxbig[:, :])
        prxlf = mp.tile([128, 16 * XG], F32)
        nc.vector.tensor_sub(prxlf[:, :], prxbig[:, :], prxh[:, :].bitcast(F32))
        prxlo = mp.tile([128, 16 * XG], F32R)
        nc.gpsimd.dma_start(prxlo[:, :], prxlf[:, :])
        nc.sync.dma_start(
            xch_stage[:, :].rearrange("p (q k) -> q p k", k=XG), prxh[:, :])
        nc.gpsimd.dma_start(
            xclo_stage[:, :].rearrange("p (q k) -> q p k", k=XG), prxlo[:, :])
        nc.scalar.dma_start(xcols[0:16, :], xch_stage[:, :])
        nc.scalar.dma_start(xcols[16:32, :], xch_stage[:, :])
        nc.scalar.dma_start(xcols[32:48, :], xclo_stage[:, :])

        # Column rows (compensated f32r, K=48):
        #   [0:16)  x_hi | y_hi, [16:32) x_hi | y_lo, [32:48) x_lo | y_hi
        # so w = x.y_hi + x_hi.y_lo + x_lo.y_hi: the f32r rounding residue
        # cancels to ~(2^-m)^2 and near-zero distances stay accurate.
        QC = M // 4
        PQ = QC // YG                  # packed partitions per quarter
        for ci in range(4):
            qsl = slice(ci * QC, (ci + 1) * QC)
            prt = slice(ci * PQ, (ci + 1) * PQ)
            # one issue per (quarter, hi/lo): the DRAM AP presents the
            # packed (q, p, k) order so the SBUF side streams linearly
            hsl = slice(ci * QC, ci * QC + QC // 2)
            hsr = slice(ci * QC + QC // 2, (ci + 1) * QC)
            pl = slice(ci * PQ, ci * PQ + PQ // 2)
            pr = slice(ci * PQ + PQ // 2, (ci + 1) * PQ)
            nc.sync.dma_start(
                ych_stage[:, hsl].rearrange("p (q k) -> q p k", k=YG),
                pryh[pl, :])
            nc.scalar.dma_start(
                ych_stage[:, hsr].rearrange("p (q k) -> q p k", k=YG),
                pryh[pr, :])
            nc.gpsimd.dma_start(
                yclo_stage[:, hsl].rearrange("p (q k) -> q p k", k=YG),
                prylo[pl, :])
            nc.gpsimd.dma_start(
                yclo_stage[:, hsr].rearrange("p (q k) -> q p k", k=YG),
                prylo[pr, :])
            nc.scalar.dma_start(ycols[0:16, qsl], ych_stage[:, qsl])
            nc.sync.dma_start(ycols[16:32, qsl], yclo_stage[:, qsl])
            nc.scalar.dma_start(ycols[32:48, qsl], ych_stage[:, qsl])



def _main_loop(nc, tc, out, ycols, xcols, tail_op):
    # Per phase (4 strips of 128 rows):
    #   [sqrt table]  per strip, per 2048-col group: 4 f32r K=16 matmuls
    #                 -> PSUM, then ACT sqrt -> strip-wide u tile (fp16)
    #   [exp table]   per strip: one strip-wide e3 = exp(-u - ln3) (fp16)
    #   DVE tail + output DMA per 2048-col group
    # Batching all 16 sqrts before the 4 exps keeps it to 2 ACT table
    # switches per phase (sqrt and exp live in different table sets).
    with tc.tile_pool(name="main_psum", bufs=2, space="PSUM") as pp, \
         tc.tile_pool(name="upool", bufs=PHASE_STRIPS + 1) as up, \
         tc.tile_pool(name="epool", bufs=PHASE_STRIPS) as ep, \
         tc.tile_pool(name="opool", bufs=3) as op_:
        ph0 = 0
        for nph in PHASE_SIZES:
            strips = range(ph0, ph0 + nph)
            ph0 += nph
            utiles = {}
            for s in strips:
                lhs = xcols[:, s * 128:(s + 1) * 128]
                u = up.tile([128, M], F16, tag="u")
                utiles[s] = u
                for g in range(N_GRP):
                    pw = pp.tile([128, GRP], F32, tag="pw")
                    for j in range(0, GRP, MMW):
                        nc.tensor.matmul(
                            pw[:, j:j + MMW],
                            lhsT=lhs,
                            rhs=ycols[:, g * GRP + j:g * GRP + j + MMW],
                            start=True, stop=True,
                        )
                    nc.scalar.activation(
                        u[:, g * GRP:(g + 1) * GRP], pw[:, :], Act.Sqrt)
            for s in strips:
                e3 = ep.tile([128, M], F16, tag="e3")
                if EXP_STRIPWISE and s != N_STRIPS - 1:
                    nc.scalar.activation(
                        e3[:, :], utiles[s][:, :], Act.Exp,
                        bias=-LN3, scale=-1.0)
                for g in range(N_GRP):
                    sl = slice(g * GRP, (g + 1) * GRP)
                    if not (EXP_STRIPWISE and s != N_STRIPS - 1):
                        nc.scalar.activation(
                            e3[:, sl], utiles[s][:, sl], Act.Exp,
                            bias=-LN3, scale=-1.0)
                    o = op_.tile([128, GRP], F32, tag="o")
                    nc.vector._custom_dve(
                        tail_op, out=o[:, :], in0=utiles[s][:, sl],
                        in1=e3[:, sl], s0=1.5, s1=0.75,
                    )
                    nc.sync.dma_start(
                        out[s * 128:(s + 1) * 128, g * GRP:(g + 1) * GRP],
                        o[:, :],
                    )


_NC_CACHE = None


def kernel(**inputs):
    global _NC_CACHE
    if _NC_CACHE is None:
        _NC_CACHE = build()
    nc = _NC_CACHE
    x = np.ascontiguousarray(np.asarray(inputs["x"], dtype=np.float32))
    base = {
        "y": np.ascontiguousarray(np.asarray(inputs["y"], dtype=np.float32)),
        "W1": np.ascontiguousarray(np.asarray(inputs["W1"], dtype=np.float32)),
        "b1": np.ascontiguousarray(np.asarray(inputs["b1"], dtype=np.float32)),
        "W2": np.ascontiguousarray(np.asarray(inputs["W2"], dtype=np.float32)),
        "b2": np.ascontiguousarray(np.asarray(inputs["b2"], dtype=np.float32)),
    }
    in_maps = [
        {"x": x[c * ROWS:(c + 1) * ROWS], **base} for c in range(N_CORES)
    ]
    res = run_bass_kernel_spmd(nc, in_maps, core_ids=list(range(N_CORES)))
    return np.concatenate([res.results[c]["out"] for c in range(N_CORES)], axis=0)
